# revision 14
# baseline (speedup 1.0000x reference)
"""Haar wavelet transform (low, high) on Trainium2, 8-core data parallel.

Input  x: (8, 64, 512, 512) f32
Output (low, high): each (8, 64, 256, 256) f32
  For 2x2 blocks [[a,b],[c,d]]:
    low  = 0.5*(a+b+c+d)
    high = lh+hl+hh = 2*d - low

Sharding: batch dim -> 1 batch element per core (no cross-core comms).

The kernel is DMA-bound (16 SDMA engines ~100% busy at their ~27GB/s
per-engine ceiling), so all device traffic runs in bf16: the host
pre-scales x by 0.5 and casts to bf16 (rel-err ~3e-3, tolerance 2e-2),
halving HBM bytes from 100.7MB to 48MB per core. With x' = x/2:
    low  = a'+b'+c'+d'
    high = 4*d' - low
low/high are stored row-interleaved in ONE dram tensor so each tile
does a single 2MB store with 16KB-per-partition descriptors; loads are
4MB tiles with 32KB descriptors (descriptor size governs per-engine
DMA rate: 32KB/16KB descriptors sustain ~26.5GB/s/engine, ~430-470GB/s
aggregate; smaller descriptors fall measurably below the roofline).

The host additionally de-interleaves even/odd COLUMNS into row halves
(row = [even cols (256) | odd cols (256)]), so every DVE op reads and
writes unit-stride bf16 and auto-selects the 2x perf mode, keeping
DVE (33us under the DMA time) off the critical path.

Per-core device program: view x' as (64*512, 512) rows; each tile is
4096 input rows -> SBUF [128 x 32*512] bf16 (32 rows per partition).
Loads on the SP HWDGE ring (4-slot ring), stores on the ACT ring
(3-slot output ring); compute on DVE:
  s    = even_rows + odd_rows          -> per row [a+c | b+d]   (2x)
  low  = s[:, :256] + s[:, 256:]                                (2x)
  high = (d' * 4) - low                (scalar_tensor_tensor, 1x)
"""

import sys

import numpy as np

for _p in ("/opt/trn_rl_repo",):
    if _p not in sys.path:
        sys.path.insert(0, _p)

# per-core problem geometry (hardcoded; one batch element per core)
_B = 8
_C, _H, _W = 64, 512, 512
_P = 128          # SBUF partitions
_R = 32           # input image rows per partition per tile
_ROWS = _C * _H   # 32768 input rows per core
_TR = _P * _R     # 4096 input rows per tile
_NT = _ROWS // _TR
_OW = _W // 2
_OROWS = _ROWS // 2
_NBUF_IN = 4      # tin ring depth
_NBUF_OUT = 3     # out ring depth
_OPS = 3          # DVE ops per tile

_prog_cache = {}


def _build_program():
    if "nc" in _prog_cache:
        return _prog_cache["nc"]
    import concourse.bass as bass
    from concourse import mybir

    bf16 = mybir.dt.bfloat16
    nc = bass.Bass()
    x = nc.declare_dram_parameter("x", [_ROWS, _W], bf16, isOutput=False)
    # low/high row-interleaved: out[r, 0, :] = low row r, out[r, 1, :] = high
    out = nc.declare_dram_parameter("out", [_OROWS, 2, _OW], bf16, isOutput=True)

    import contextlib

    with contextlib.ExitStack() as ctx:
        tin = [
            ctx.enter_context(
                nc.sbuf_tensor(f"tin{k}", [_P, _R * _W], bf16)
            )
            for k in range(_NBUF_IN)
        ]
        s = ctx.enter_context(
            nc.sbuf_tensor("s", [_P, (_R // 2) * _W], bf16)
        )
        ob = [
            ctx.enter_context(
                nc.sbuf_tensor(f"ob{k}", [_P, (_R // 2) * 2 * _OW], bf16)
            )
            for k in range(_NBUF_OUT)
        ]
        # Per-ring-slot DMA sems: a slot's next DMA only dispatches after
        # the previous one was consumed, so "slot sem >= 16*count" exactly
        # means "all of this slot's DMAs landed on every SDMA engine".
        load_sem = [
            ctx.enter_context(nc.semaphore(f"load_sem{k}"))
            for k in range(_NBUF_IN)
        ]
        st_out = [
            ctx.enter_context(nc.semaphore(f"st_out{k}"))
            for k in range(_NBUF_OUT)
        ]
        dve_done = ctx.enter_context(nc.semaphore("dve_done"))
        block = ctx.enter_context(nc.Block())

        def in_src(i):
            return x[i * _TR : (i + 1) * _TR, :].rearrange(
                "(p r) w -> p (r w)", p=_P
            )

        def out_dst(i):
            orows = _TR // 2
            return out[i * orows : (i + 1) * orows, :, :].rearrange(
                "(p r) t j -> p (r t j)", p=_P
            )

        @block.sync
        def _(sync):
            # loads on the SP HWDGE ring
            for i in range(min(_NBUF_IN, _NT)):
                sync.dma_start(tin[i][:], in_src(i)).then_inc(
                    load_sem[i % _NBUF_IN], 16
                )
            for i in range(_NT - _NBUF_IN):
                # tin slot (i % NBUF) is free once iter i's last reader
                # (the STT high op, 3rd DVE op of iter i) retired
                sync.wait_ge(dve_done, _OPS * (i + 1))
                j = i + _NBUF_IN
                sync.dma_start(tin[j % _NBUF_IN][:], in_src(j)).then_inc(
                    load_sem[j % _NBUF_IN], 16
                )

        @block.vector
        def _(vector):
            for i in range(_NT):
                vector.wait_ge(load_sem[i % _NBUF_IN], 16 * (i // _NBUF_IN + 1))
                if i >= _NBUF_OUT:
                    # out slot reuse: store of iter i-NBUF_OUT done
                    vector.wait_ge(st_out[i % _NBUF_OUT], 16 * (i // _NBUF_OUT))
                tb = tin[i % _NBUF_IN]
                t3in = tb[:].rearrange("p (r w) -> p r w", w=_W)
                ev = t3in[:, 0::2, :]
                od = t3in[:, 1::2, :]
                # odd rows, odd cols = right half of od (host de-interleave)
                d = t3in[:, 1::2, _OW:]
                s3 = s[:].rearrange("p (k w) -> p k w", w=_W)
                o3 = ob[i % _NBUF_OUT][:].rearrange(
                    "p (k w) -> p k w", w=2 * _OW
                )
                lo3 = o3[:, :, :_OW]
                hi3 = o3[:, :, _OW:]
                nc.vector.tensor_add(s3, ev, od).then_inc(dve_done, 1)
                nc.vector.tensor_add(
                    lo3, s3[:, :, :_OW], s3[:, :, _OW:]
                ).then_inc(dve_done, 1)
                nc.vector.scalar_tensor_tensor(
                    hi3, d, 4.0, lo3,
                    mybir.AluOpType.mult, mybir.AluOpType.subtract,
                ).then_inc(dve_done, 1)

        @block.scalar
        def _(scalar):
            # stores on the ACT HWDGE ring
            for i in range(_NT):
                scalar.wait_ge(dve_done, _OPS * i + 3)
                scalar.dma_start(out_dst(i), ob[i % _NBUF_OUT][:]).then_inc(
                    st_out[i % _NBUF_OUT], 16
                )
            # final: all stores landed
            for k in range(_NBUF_OUT):
                nslot = (_NT - 1 - k) // _NBUF_OUT + 1
                scalar.wait_ge(st_out[k], 16 * nslot)

    _prog_cache["nc"] = nc
    return nc


def _run(x: np.ndarray, trace: bool = False):
    import ml_dtypes

    from concourse.bass_utils import run_bass_kernel_spmd

    nc = _build_program()
    xs = np.asarray(x).reshape(_B, _ROWS, _W)
    assert xs.shape == (_B, _ROWS, _W), xs.shape
    # fold the Haar 0.5 into a host-side pre-scale, cast to bf16, and
    # de-interleave even/odd columns into row halves so the device ops
    # are all unit-stride
    half = np.float32(0.5)
    xh = np.empty((_B, _ROWS, _W), dtype=ml_dtypes.bfloat16)
    xh[:, :, : _OW] = xs[:, :, 0::2] * half
    xh[:, :, _OW :] = xs[:, :, 1::2] * half
    in_maps = [{"x": xh[b]} for b in range(_B)]
    out = run_bass_kernel_spmd(nc, in_maps, list(range(_B)), trace=trace)
    lows, highs = [], []
    for b in range(_B):
        ob = np.asarray(out.results[b]["out"], dtype=np.float32).reshape(
            _C, _H // 2, 2, _W // 2
        )
        lows.append(ob[:, :, 0, :])
        highs.append(ob[:, :, 1, :])
    return (np.stack(lows), np.stack(highs)), out


def kernel(x: np.ndarray):
    (low, high), _ = _run(x, trace=False)
    return low, high
